# revision 10
# baseline (speedup 1.0000x reference)
"""Multi-head attention (batch=2, seq=2048, dim=256, nhead=8, head_dim=256)
distributed across 8 trn2 NeuronCores.

Sharding: the 16 (batch, head) pairs are distributed 2-per-core (cores 0-3
handle batch 0 heads 0-7, cores 4-7 batch 1). The host sums the 4 partials
per batch and adds the output bias.

Per-head math is restructured to cut PE work:
  scores = q k^T / 16 = x (Wq_h^T Wk_h / 16) x^T = x A_h x^T
  out_h  = softmax(scores) (x (Wo_h Wv_h)^T)     = W x C_h^T
A_h (fp8, pre-scaled by 2^13) and C_h^T (bf16) are precomputed on the host,
eliminating the separate q/k projections and the entire Wo stage.

On-device per core:
  xf8 (fp8 x^T, DoubleRow ko-stacked) is shipped from host. qaT = (xA)^T via
  fp8-DR matmul, evicted fp8. QK = qaT x fp8-DR (contraction 256, one pass).
  E = exp(scores * 2^-13) via ScalarE straight out of PSUM. AV runs bf16 with
  lhsT = E sq-slices so the output is [sq-part, o-cols]; v'2 carries a ones
  column per (kt, head) so the AV psum's last column accumulates the softmax
  denominator -- per-partition reciprocal fused into the eviction. No
  denominator tree, no Wo matmuls.
"""

import sys

if "/opt/trn_rl_repo" not in sys.path:
    sys.path.insert(0, "/opt/trn_rl_repo")

import numpy as np
import ml_dtypes

P = 128
S = 2048
D = 256
CHUNK = 512
CH = S // CHUNK  # 4 sq chunks
NKT = S // P     # 16 sk tiles
NHEAD = 8
NCORES = 8
ASCALE = 2.0 ** 11  # pre-scale on A_h so fp8 quantization avoids subnormals

_BUILT = None


def _build():
    import concourse.bacc as bacc
    import concourse.mybir as mybir
    import concourse.tile as tile
    from contextlib import ExitStack

    BF = mybir.dt.bfloat16
    FP8 = mybir.dt.float8e4
    F32 = mybir.dt.float32
    EXP = mybir.ActivationFunctionType.Exp
    DR = mybir.MatmulPerfMode.DoubleRow
    VW = 2 * D + 2  # 514: per-kt width of v'2 (2 heads x (256 + ones col))

    nc = bacc.Bacc(None, target_bir_lowering=False, debug=False)
    with tile.TileContext(nc) as tc:
        with ExitStack() as ctx:
            dram = ctx.enter_context(tc.tile_pool(name="dram", bufs=1, space="DRAM"))
            xt_d = dram.tile([2, P, S], BF, kind="ExternalInput", name="xt")
            xf8_d = dram.tile([P, 2, S], FP8, kind="ExternalInput", name="xf8")
            a_d = dram.tile([2, P, 2, D], FP8, kind="ExternalInput", name="a")
            c2_d = dram.tile([2, P, 2 * D], BF, kind="ExternalInput", name="c2")
            out_d = dram.tile([S, D], F32, kind="ExternalOutput", name="out")

            const = ctx.enter_context(tc.tile_pool(name="const", bufs=1))
            dum_in = const.tile([P, 1], BF, name="dum_in")
            dum_out = const.tile([P, 1], BF, name="dum_out")
            nc.vector.memset(dum_in[:], 0.0)

            xpool = ctx.enter_context(tc.tile_pool(name="xtp", bufs=1))
            wpool = ctx.enter_context(tc.tile_pool(name="wp", bufs=1))
            xt_sb = [xpool.tile([P, S], BF, name=f"xt{et}") for et in range(2)]
            xf8_sb = xpool.tile([P, 2 * S], FP8, name="xf8")
            a_sb = [wpool.tile([P, 2 * D], FP8, name=f"a{j}") for j in range(2)]
            c2_sb = [wpool.tile([P, 2 * D], BF, name=f"c2{et}") for et in range(2)]

            # ---- input DMAs: few large pieces, priority order (first compute
            # needs xf8 + A0). Scalar issues only the two late xf8 halves,
            # queued before its Exp table load so they go out immediately.
            dma_engines = [nc.sync, nc.gpsimd]
            H = S // 2

            def xf8_piece(ko, half):
                return (xf8_sb[:, ko * S + half * H: ko * S + (half + 1) * H],
                        xf8_d[:, ko, half * H:(half + 1) * H])

            for ko in range(2):
                dst, src = xf8_piece(ko, 1)
                nc.scalar.dma_start(out=dst, in_=src)
            # warm the ScalarE Exp table (~2.7us) during the input DMAs
            nc.scalar.activation(dum_out[:], dum_in[:], EXP)

            queues = {0: [], 1: []}
            for ko in range(2):
                queues[ko].append(xf8_piece(ko, 0))
            for j in range(2):
                queues[j].append((a_sb[j][:], a_d[j].rearrange("p ko d -> p (ko d)")))
            for half in range(2):
                for et in range(2):
                    queues[et].append((xt_sb[et][:, half * H:(half + 1) * H],
                                       xt_d[et, :, half * H:(half + 1) * H]))
            for et in range(2):
                queues[et].append((c2_sb[et][:], c2_d[et]))
            for q, ldl in queues.items():
                for dst, srcap in ldl:
                    dma_engines[q].dma_start(out=dst, in_=srcap)

            xf83 = xf8_sb.rearrange("p (ko s) -> p ko s", ko=2)
            a3 = [a_sb[j].rearrange("p (ko d) -> p ko d", ko=2) for j in range(2)]

            vpool = ctx.enter_context(tc.tile_pool(name="vp", bufs=1))
            v2_sb = vpool.tile([P, NKT * VW], BF, name="v2")
            v23 = v2_sb.rearrange("p (k x) -> p k x", k=NKT)
            # ones columns (denominator accumulators) at j*(D+1)+D per kt block
            nc.vector.memset(v23[:, :, D:D + 1], 1.0)
            nc.vector.memset(v23[:, :, 2 * D + 1:2 * D + 2], 1.0)

            qapool = ctx.enter_context(tc.tile_pool(name="qap", bufs=2))
            epool = ctx.enter_context(tc.tile_pool(name="ep", bufs=3))
            rpool = ctx.enter_context(tc.tile_pool(name="rp", bufs=4))
            fpool = ctx.enter_context(tc.tile_pool(name="fp", bufs=1))
            final_sb = fpool.tile([P, NKT * D], F32, name="final")

            psA = ctx.enter_context(tc.tile_pool(name="psA", bufs=2, space="PSUM"))
            psB = ctx.enter_context(tc.tile_pool(name="psB", bufs=3, space="PSUM"))

            # ---- v' projection for BOTH heads: v'2[sk, kt-blocks of
            # [h0 256 | 1 | h1 256 | 1]] -- the ones columns make the AV psum
            # accumulate the softmax denominator for free.
            def emit_vproj():
                for st in range(NKT):
                    ps = psB.tile([P, CHUNK], F32, tag="psB", name="ps_v")
                    for et in range(2):
                        nc.tensor.matmul(
                            ps[:],
                            lhsT=xt_sb[et][:, st * P:(st + 1) * P],
                            rhs=c2_sb[et][:],
                            start=(et == 0), stop=(et == 1),
                        )
                    dst = v2_sb[:, st * VW: st * VW + VW].rearrange(
                        "p (h x) -> p h x", h=2)[:, :, 0:D]
                    nc.vector.tensor_copy(dst, ps[:].rearrange("p (h x) -> p h x", h=2))

            # ---- qa projection: (x A_h)^T [a=256, s], fp8 out, DR layout.
            def emit_qa(j, qa_sb, cs):
                for c in cs:
                    for dt in range(2):
                        ps = psB.tile([P, CHUNK], F32, tag="psB", name="ps_qa")
                        nc.tensor.matmul(
                            ps[:],
                            lhsT=a3[j][:, :, dt * P:(dt + 1) * P],
                            rhs=xf83[:, :, c * CHUNK:(c + 1) * CHUNK],
                            start=True, stop=True, perf_mode=DR,
                        )
                        nc.vector.tensor_copy(
                            qa_sb[:, dt * S + c * CHUNK: dt * S + (c + 1) * CHUNK],
                            ps[:])

            # ---- QK for chunk c: scores[sk, sq-chunk], fp8 DR; exp straight
            # out of PSUM (scaled scores |s| < ~1).
            def emit_qk(j, qa3, c):
                E = epool.tile([P, NKT * CHUNK], BF, tag="E", name=f"E_{j}_{c}")
                for g in range(NKT // 2):
                    ps = psA.tile([P, 2 * CHUNK], F32, tag="psA", name="ps_qk")
                    for half in range(2):
                        kt = 2 * g + half
                        nc.tensor.matmul(
                            ps[:, half * CHUNK:(half + 1) * CHUNK],
                            lhsT=xf83[:, :, kt * P:(kt + 1) * P],
                            rhs=qa3[:, :, c * CHUNK:(c + 1) * CHUNK],
                            start=True, stop=True, perf_mode=DR,
                        )
                    nc.scalar.activation(
                        E[:, g * 2 * CHUNK:(g + 1) * 2 * CHUNK], ps[:],
                        EXP, scale=1.0 / ASCALE,
                    )
                return E

            # ---- AV for chunk c: out[sq-tile, 256 o-cols + denom col].
            # lhsT = E sq-slices, rhs = v'2 (with ones col). The reciprocal of
            # the accumulated denominator is fused into the eviction; head 1
            # adds onto head 0's partial and streams the result to DRAM.
            def emit_av(j, E, c):
                for st in range(CHUNK // P):
                    gst = c * (CHUNK // P) + st
                    ps = psB.tile([P, CHUNK], F32, tag="psB", name="ps_av")
                    for kt in range(NKT):
                        nc.tensor.matmul(
                            ps[:, 0:D + 1],
                            lhsT=E[:, kt * CHUNK + st * P: kt * CHUNK + (st + 1) * P],
                            rhs=v2_sb[:, kt * VW + j * (D + 1): kt * VW + j * (D + 1) + D + 1],
                            start=(kt == 0), stop=(kt == NKT - 1),
                        )
                    recip = rpool.tile([P, 1], F32, tag="r", name="recip")
                    nc.vector.reciprocal(recip[:], ps[:, D:D + 1])
                    if j == 0:
                        nc.vector.tensor_scalar_mul(
                            final_sb[:, gst * D:(gst + 1) * D], ps[:, 0:D], recip[:])
                    else:
                        nc.vector.scalar_tensor_tensor(
                            final_sb[:, gst * D:(gst + 1) * D],
                            ps[:, 0:D], recip[:],
                            final_sb[:, gst * D:(gst + 1) * D],
                            op0=mybir.AluOpType.mult, op1=mybir.AluOpType.add,
                        )
                        dma_engines[gst % 2].dma_start(
                            out=out_d[gst * P:(gst + 1) * P, :],
                            in_=final_sb[:, gst * D:(gst + 1) * D],
                        )

            qa_sb = [qapool.tile([P, 2 * S], FP8, tag="qa", name=f"qa{j}")
                     for j in range(2)]
            qa3 = [qa_sb[j].rearrange("p (ko s) -> p ko s", ko=2) for j in range(2)]

            # ---- schedule: chunk-skewed pipeline (QK of c+1 issued before
            # AV of c so ScalarE exp overlaps PE work); qa projections and
            # v'proj run while the input DMAs stream and exp(h0,c0) drains.
            emit_qa(0, qa_sb[0], [0, 1, 2, 3])
            E_prev = emit_qk(0, qa3[0], 0)
            emit_vproj()
            emit_qa(1, qa_sb[1], [0, 1, 2, 3])
            for c in range(1, CH):
                E_c = emit_qk(0, qa3[0], c)
                emit_av(0, E_prev, c - 1)
                E_prev = E_c
            E_h1 = emit_qk(1, qa3[1], 0)
            emit_av(0, E_prev, CH - 1)
            E_prev = E_h1
            for c in range(1, CH):
                E_c = emit_qk(1, qa3[1], c)
                emit_av(1, E_prev, c - 1)
                E_prev = E_c
            emit_av(1, E_prev, CH - 1)
    nc.compile()
    names = dict(xt=xt_d.name, xf8=xf8_d.name, a=a_d.name, c2=c2_d.name,
                 out=out_d.name)
    return nc, names


def _get_built():
    global _BUILT
    if _BUILT is None:
        _BUILT = _build()
    return _BUILT


def _prep_core_inputs(i, x, Wq, Wk, Wv, Wo, names):
    bf16 = ml_dtypes.bfloat16
    fp8 = ml_dtypes.float8_e4m3
    b = i // 4
    heads = [(2 * i) % NHEAD, (2 * i) % NHEAD + 1]

    xbT = np.ascontiguousarray(x[b].T)                      # [d=256, s]
    xt = xbT.reshape(2, P, S).astype(bf16)                  # [et, 128, s]
    xf8 = np.ascontiguousarray(
        xbT.reshape(2, P, S).transpose(1, 0, 2)).astype(fp8)  # [ki, ko, s]

    a_list, ct_list = [], []
    for h in heads:
        Wq_h = Wq[h * D:(h + 1) * D, :]
        Wk_h = Wk[h * D:(h + 1) * D, :]
        Wv_h = Wv[h * D:(h + 1) * D, :]
        Wo_h = Wo[:, h * D:(h + 1) * D]
        A = (Wq_h.T @ Wk_h) * (ASCALE / (D ** 0.5))          # [d_in, d_in']
        a_list.append(A.reshape(2, P, D).transpose(1, 0, 2))  # [ki, ko, a]
        ct_list.append((Wo_h @ Wv_h).T)                       # C^T [d_in, o]
    a_arr = np.stack(a_list).astype(fp8)                      # [j, ki, ko, a]
    c2 = np.concatenate(ct_list, axis=1).reshape(2, P, 2 * D).astype(bf16)
    return {names["xt"]: xt, names["xf8"]: xf8, names["a"]: a_arr,
            names["c2"]: c2}


def kernel(x, Wq, Wk, Wv, Wo, bo):
    from concourse.bass_utils import run_bass_kernel_spmd

    x = np.asarray(x, dtype=np.float32)
    Wq = np.asarray(Wq, dtype=np.float32)
    Wk = np.asarray(Wk, dtype=np.float32)
    Wv = np.asarray(Wv, dtype=np.float32)
    Wo = np.asarray(Wo, dtype=np.float32)
    bo = np.asarray(bo, dtype=np.float32)

    nc, names = _get_built()
    in_maps = [_prep_core_inputs(i, x, Wq, Wk, Wv, Wo, names) for i in range(NCORES)]
    res = run_bass_kernel_spmd(nc, in_maps, core_ids=list(range(NCORES)))

    out = np.zeros((2, S, D), dtype=np.float32)
    for b in range(2):
        acc = np.zeros((S, D), dtype=np.float32)
        for i in range(4 * b, 4 * b + 4):
            acc += res.results[i][names["out"]]
        out[b] = acc + bo[None, :]
    return out


# revision 13
# speedup vs baseline: 1.1716x; 1.1716x over previous
"""Multi-head attention (batch=2, seq=2048, dim=256, nhead=8, head_dim=256)
distributed across 8 trn2 NeuronCores.

Sharding: the 16 (batch, head) pairs are distributed 2-per-core (cores 0-3
handle batch 0 heads 0-7, cores 4-7 batch 1). The host sums the 4 partials
per batch and adds the output bias.

Per-head math is restructured to cut PE work:
  scores = q k^T / 16 = x (Wq_h^T Wk_h / 16) x^T = x A_h x^T
  out_h  = softmax(scores) (x (Wo_h Wv_h)^T)     = W x C_h^T
A_h (fp8, pre-scaled by 2^13) and C_h^T (bf16) are precomputed on the host,
eliminating the separate q/k projections and the entire Wo stage.

On-device per core:
  xf8 (fp8 x^T, DoubleRow ko-stacked) is shipped from host. qaT = (xA)^T via
  fp8-DR matmul, evicted fp8. QK = qaT x fp8-DR (contraction 256, one pass).
  E = exp(scores * 2^-13) via ScalarE straight out of PSUM. AV runs bf16 with
  lhsT = E sq-slices so the output is [sq-part, o-cols]; v'2 carries a ones
  column per (kt, head) so the AV psum's last column accumulates the softmax
  denominator -- per-partition reciprocal fused into the eviction. No
  denominator tree, no Wo matmuls.
"""

import sys

if "/opt/trn_rl_repo" not in sys.path:
    sys.path.insert(0, "/opt/trn_rl_repo")

import numpy as np
import ml_dtypes

P = 128
S = 2048
D = 256
CHUNK = 512
CH = S // CHUNK  # 4 sq chunks
NKT = S // P     # 16 sk tiles
NHEAD = 8
NCORES = 8
ASCALE = 2.0 ** 11  # pre-scale on A_h so fp8 quantization avoids subnormals

_BUILT = None


def _build():
    import concourse.bacc as bacc
    import concourse.mybir as mybir
    import concourse.tile as tile
    from contextlib import ExitStack

    BF = mybir.dt.bfloat16
    FP8 = mybir.dt.float8e4
    F32 = mybir.dt.float32
    EXP = mybir.ActivationFunctionType.Exp
    DR = mybir.MatmulPerfMode.DoubleRow
    VW = 2 * D + 2  # 514: per-kt width of v'2 (2 heads x (256 + ones col))

    nc = bacc.Bacc(None, target_bir_lowering=False, debug=False)
    with tile.TileContext(nc) as tc:
        with ExitStack() as ctx:
            dram = ctx.enter_context(tc.tile_pool(name="dram", bufs=1, space="DRAM"))
            xt_d = dram.tile([2, P, S], BF, kind="ExternalInput", name="xt")
            xf8_d = dram.tile([P, 2, S], FP8, kind="ExternalInput", name="xf8")
            a_d = dram.tile([2, P, 2, D], FP8, kind="ExternalInput", name="a")
            c2_d = dram.tile([2, P, 2 * D], BF, kind="ExternalInput", name="c2")
            out_d = dram.tile([S, D], F32, kind="ExternalOutput", name="out")

            const = ctx.enter_context(tc.tile_pool(name="const", bufs=1))
            dum_in = const.tile([P, 1], BF, name="dum_in")
            dum_out = const.tile([P, 1], BF, name="dum_out")
            nc.vector.memset(dum_in[:], 0.0)

            xpool = ctx.enter_context(tc.tile_pool(name="xtp", bufs=1))
            wpool = ctx.enter_context(tc.tile_pool(name="wp", bufs=1))
            xt_sb = [xpool.tile([P, S], BF, name=f"xt{et}") for et in range(2)]
            xf8_sb = xpool.tile([P, 2 * S], FP8, name="xf8")
            a_sb = [wpool.tile([P, 2 * D], FP8, name=f"a{j}") for j in range(2)]
            c2_sb = [wpool.tile([P, 2 * D], BF, name=f"c2{et}") for et in range(2)]

            # ---- input DMAs: 3 HWDGE/SWDGE rings at ~46 GB/s each; balance
            # ~0.64 MB per ring with the critical pieces (xf8 chunks, A)
            # first and xt streaming behind (v'proj is scheduled late).
            # Scalar's pieces are queued before its Exp table load.
            H = S // 2

            def xf8_piece(ko, half):
                return (xf8_sb[:, ko * S + half * H: ko * S + (half + 1) * H],
                        xf8_d[:, ko, half * H:(half + 1) * H])

            ring_sync = [xf8_piece(0, 0), xf8_piece(0, 1),
                         (xt_sb[0][:, 0:H], xt_d[0, :, 0:H]),
                         (c2_sb[0][:], c2_d[0])]
            ring_scalar = [xf8_piece(1, 0), xf8_piece(1, 1),
                           (xt_sb[0][:, H:S], xt_d[0, :, H:S]),
                           (c2_sb[1][:], c2_d[1])]
            ring_gpsimd = [(a_sb[0][:], a_d[0].rearrange("p ko d -> p (ko d)")),
                           (a_sb[1][:], a_d[1].rearrange("p ko d -> p (ko d)")),
                           (xt_sb[1][:, 0:H], xt_d[1, :, 0:H]),
                           (xt_sb[1][:, H:S], xt_d[1, :, H:S])]
            for dst, srcap in ring_scalar:
                nc.scalar.dma_start(out=dst, in_=srcap)
            # warm the ScalarE Exp table (~2.7us) during the input DMAs
            nc.scalar.activation(dum_out[:], dum_in[:], EXP)
            for dst, srcap in ring_sync:
                nc.sync.dma_start(out=dst, in_=srcap)
            for dst, srcap in ring_gpsimd:
                nc.gpsimd.dma_start(out=dst, in_=srcap)
            dma_engines = [nc.sync, nc.gpsimd]

            xf83 = xf8_sb.rearrange("p (ko s) -> p ko s", ko=2)
            a3 = [a_sb[j].rearrange("p (ko d) -> p ko d", ko=2) for j in range(2)]

            vpool = ctx.enter_context(tc.tile_pool(name="vp", bufs=1))
            v2_sb = vpool.tile([P, NKT * VW], BF, name="v2")
            v23 = v2_sb.rearrange("p (k x) -> p k x", k=NKT)
            # ones columns (denominator accumulators) at j*(D+1)+D per kt block
            nc.vector.memset(v23[:, :, D:D + 1], 1.0)
            nc.vector.memset(v23[:, :, 2 * D + 1:2 * D + 2], 1.0)

            qapool = ctx.enter_context(tc.tile_pool(name="qap", bufs=2))
            epool = ctx.enter_context(tc.tile_pool(name="ep", bufs=3))
            rpool = ctx.enter_context(tc.tile_pool(name="rp", bufs=4))
            fpool = ctx.enter_context(tc.tile_pool(name="fp", bufs=1))
            final_sb = fpool.tile([P, NKT * D], F32, name="final")

            psA = ctx.enter_context(tc.tile_pool(name="psA", bufs=2, space="PSUM"))
            psB = ctx.enter_context(tc.tile_pool(name="psB", bufs=3, space="PSUM"))

            # ---- v' projection for BOTH heads: v'2[sk, kt-blocks of
            # [h0 256 | 1 | h1 256 | 1]] -- the ones columns make the AV psum
            # accumulate the softmax denominator for free.
            def emit_vproj():
                for st in range(NKT):
                    ps = psB.tile([P, CHUNK], F32, tag="psB", name="ps_v")
                    for et in range(2):
                        nc.tensor.matmul(
                            ps[:],
                            lhsT=xt_sb[et][:, st * P:(st + 1) * P],
                            rhs=c2_sb[et][:],
                            start=(et == 0), stop=(et == 1),
                        )
                    dst = v2_sb[:, st * VW: st * VW + VW].rearrange(
                        "p (h x) -> p h x", h=2)[:, :, 0:D]
                    nc.vector.tensor_copy(dst, ps[:].rearrange("p (h x) -> p h x", h=2))

            # ---- qa projection: (x A_h)^T [a=256, s], fp8 out, DR layout.
            def emit_qa(j, qa_sb, cs):
                for c in cs:
                    for dt in range(2):
                        ps = psB.tile([P, CHUNK], F32, tag="psB", name="ps_qa")
                        nc.tensor.matmul(
                            ps[:],
                            lhsT=a3[j][:, :, dt * P:(dt + 1) * P],
                            rhs=xf83[:, :, c * CHUNK:(c + 1) * CHUNK],
                            start=True, stop=True, perf_mode=DR,
                        )
                        nc.vector.tensor_copy(
                            qa_sb[:, dt * S + c * CHUNK: dt * S + (c + 1) * CHUNK],
                            ps[:])

            # ---- QK for chunk c: scores[sk, sq-chunk], fp8 DR; exp straight
            # out of PSUM (scaled scores |s| < ~1).
            def emit_qk(j, qa3, c):
                E = epool.tile([P, NKT * CHUNK], BF, tag="E", name=f"E_{j}_{c}")
                for g in range(NKT // 2):
                    ps = psA.tile([P, 2 * CHUNK], F32, tag="psA", name="ps_qk")
                    for half in range(2):
                        kt = 2 * g + half
                        nc.tensor.matmul(
                            ps[:, half * CHUNK:(half + 1) * CHUNK],
                            lhsT=xf83[:, :, kt * P:(kt + 1) * P],
                            rhs=qa3[:, :, c * CHUNK:(c + 1) * CHUNK],
                            start=True, stop=True, perf_mode=DR,
                        )
                    nc.scalar.activation(
                        E[:, g * 2 * CHUNK:(g + 1) * 2 * CHUNK], ps[:],
                        EXP, scale=1.0 / ASCALE,
                    )
                return E

            # ---- AV for chunk c: out[sq-tile, 256 o-cols + denom col].
            # lhsT = E sq-slices, rhs = v'2 (with ones col). The reciprocal of
            # the accumulated denominator is fused into the eviction; head 1
            # adds onto head 0's partial and streams the result to DRAM.
            def emit_av(j, E, c):
                for st in range(CHUNK // P):
                    gst = c * (CHUNK // P) + st
                    ps = psB.tile([P, CHUNK], F32, tag="psB", name="ps_av")
                    for kt in range(NKT):
                        nc.tensor.matmul(
                            ps[:, 0:D + 1],
                            lhsT=E[:, kt * CHUNK + st * P: kt * CHUNK + (st + 1) * P],
                            rhs=v2_sb[:, kt * VW + j * (D + 1): kt * VW + j * (D + 1) + D + 1],
                            start=(kt == 0), stop=(kt == NKT - 1),
                        )
                    recip = rpool.tile([P, 1], F32, tag="r", name="recip")
                    nc.vector.reciprocal(recip[:], ps[:, D:D + 1])
                    if j == 0:
                        nc.vector.tensor_scalar_mul(
                            final_sb[:, gst * D:(gst + 1) * D], ps[:, 0:D], recip[:])
                    else:
                        nc.vector.scalar_tensor_tensor(
                            final_sb[:, gst * D:(gst + 1) * D],
                            ps[:, 0:D], recip[:],
                            final_sb[:, gst * D:(gst + 1) * D],
                            op0=mybir.AluOpType.mult, op1=mybir.AluOpType.add,
                        )
                        dma_engines[gst % 2].dma_start(
                            out=out_d[gst * P:(gst + 1) * P, :],
                            in_=final_sb[:, gst * D:(gst + 1) * D],
                        )

            qa_sb = [qapool.tile([P, 2 * S], FP8, tag="qa", name=f"qa{j}")
                     for j in range(2)]
            qa3 = [qa_sb[j].rearrange("p (ko s) -> p ko s", ko=2) for j in range(2)]

            # ---- schedule: chunk-skewed pipeline (QK runs 2 chunks ahead of
            # AV; ScalarE exp overlaps PE work). qa projections for both
            # heads and v'proj are placed so they fill the PE while the xt
            # DMAs stream and the first exps drain.
            emit_qa(0, qa_sb[0], [0, 1, 2, 3])
            E0 = emit_qk(0, qa3[0], 0)
            emit_qa(1, qa_sb[1], [0, 1, 2, 3])
            E1 = emit_qk(0, qa3[0], 1)
            emit_vproj()
            Es = [E0, E1]
            for step in range(2, 10):
                if step < 8:  # chunks h0: c2, c3 then h1: c0..c3
                    j_qk, c_qk = divmod(step, CH)
                    Es.append(emit_qk(j_qk, qa3[j_qk], c_qk))
                j_av, c_av = divmod(step - 2, CH)
                emit_av(j_av, Es[step - 2], c_av)
                Es[step - 2] = None
    nc.compile()
    names = dict(xt=xt_d.name, xf8=xf8_d.name, a=a_d.name, c2=c2_d.name,
                 out=out_d.name)
    return nc, names


def _get_built():
    global _BUILT
    if _BUILT is None:
        _BUILT = _build()
    return _BUILT


def _prep_core_inputs(i, x, Wq, Wk, Wv, Wo, names):
    bf16 = ml_dtypes.bfloat16
    fp8 = ml_dtypes.float8_e4m3
    b = i // 4
    heads = [(2 * i) % NHEAD, (2 * i) % NHEAD + 1]

    xbT = np.ascontiguousarray(x[b].T)                      # [d=256, s]
    xt = xbT.reshape(2, P, S).astype(bf16)                  # [et, 128, s]
    xf8 = np.ascontiguousarray(
        xbT.reshape(2, P, S).transpose(1, 0, 2)).astype(fp8)  # [ki, ko, s]

    a_list, ct_list = [], []
    for h in heads:
        Wq_h = Wq[h * D:(h + 1) * D, :]
        Wk_h = Wk[h * D:(h + 1) * D, :]
        Wv_h = Wv[h * D:(h + 1) * D, :]
        Wo_h = Wo[:, h * D:(h + 1) * D]
        A = (Wq_h.T @ Wk_h) * (ASCALE / (D ** 0.5))          # [d_in, d_in']
        a_list.append(A.reshape(2, P, D).transpose(1, 0, 2))  # [ki, ko, a]
        ct_list.append((Wo_h @ Wv_h).T)                       # C^T [d_in, o]
    a_arr = np.stack(a_list).astype(fp8)                      # [j, ki, ko, a]
    c2 = np.concatenate(ct_list, axis=1).reshape(2, P, 2 * D).astype(bf16)
    return {names["xt"]: xt, names["xf8"]: xf8, names["a"]: a_arr,
            names["c2"]: c2}


def kernel(x, Wq, Wk, Wv, Wo, bo):
    from concourse.bass_utils import run_bass_kernel_spmd

    x = np.asarray(x, dtype=np.float32)
    Wq = np.asarray(Wq, dtype=np.float32)
    Wk = np.asarray(Wk, dtype=np.float32)
    Wv = np.asarray(Wv, dtype=np.float32)
    Wo = np.asarray(Wo, dtype=np.float32)
    bo = np.asarray(bo, dtype=np.float32)

    nc, names = _get_built()
    in_maps = [_prep_core_inputs(i, x, Wq, Wk, Wv, Wo, names) for i in range(NCORES)]
    res = run_bass_kernel_spmd(nc, in_maps, core_ids=list(range(NCORES)))

    out = np.zeros((2, S, D), dtype=np.float32)
    for b in range(2):
        acc = np.zeros((S, D), dtype=np.float32)
        for i in range(4 * b, 4 * b + 4):
            acc += res.results[i][names["out"]]
        out[b] = acc + bo[None, :]
    return out
